# revision 1
# baseline (speedup 1.0000x reference)
"""Trainium2 Bass kernel for nn_NodeEdgeConv (GNN message passing).

Strategy (edge-parallel, per sharding hint):
- Algebraic reduction: since segment_sum(h[idx]*msg, idx)[n] = h[n]*segment_sum(msg, idx)[n]
  and segment_sum(v @ W + b) = segment_sum(v) @ W + count*b, only the [E, 64]
  edge payloads need a device-side segment sum; all matmuls collapse to
  node-level [N,*] GEMMs.
- Each core takes 1/8 of the edges of both edge types, scatter-adds its
  v-rows into a combined [2N, 64] table with dma_scatter_add. Duplicate
  indices within one scatter instruction lose updates on HW (SDMA RMW races),
  so edges are scheduled into conflict-free "waves" (occurrence rank of the
  destination row); each instruction only carries distinct rows. Padding
  tokens target dummy rows past the table end.
- One ReduceScatter over the table gives each core its node shard of both
  aggregates; the per-node finish (Linear+LayerNorm+Linear residual) runs on
  the shard; host concatenates the 8 output shards.
"""

import numpy as np

import concourse.bass as bass
import concourse.bacc as bacc
import concourse.mybir as mybir
import concourse.tile as tile
from concourse.masks import make_identity

F32 = mybir.dt.float32
I16 = mybir.dt.int16


class Cfg:
    def __init__(self, n_nodes=8192, e=524288, d=128, m=64, n_cores=8, tpi=4096):
        self.N = n_nodes          # nodes per side
        self.E = e                # edges per type (total)
        self.D = d
        self.M = m
        self.C = n_cores
        self.TPI = tpi
        self.EL = e // n_cores    # edges per core per type
        self.NSH = n_nodes // n_cores   # nodes per core per side
        self.NV = 2 * n_nodes     # virtual table rows
        self.TROWS = self.NV + 128  # + dummy rows


# ---------------- host-side schedule ----------------

def occurrence_rank(virt):
    """occ[i] = number of j<i with virt[j]==virt[i]."""
    order = np.argsort(virt, kind="stable")
    sv = virt[order]
    is_new = np.r_[True, sv[1:] != sv[:-1]]
    run_starts = np.flatnonzero(is_new)
    run_id = np.cumsum(is_new) - 1
    occ_sorted = np.arange(len(sv)) - run_starts[run_id]
    occ = np.empty_like(occ_sorted)
    occ[order] = occ_sorted
    return occ


def core_wave_streams(idxA, idxB, cfg):
    """Per core: edge stream in wave-major order + per-wave sizes.
    Returns (stream_edges [EL*2] int64 global edge ids with type tag,
             wave_sizes list). Edge id: type A edge k -> k; type B -> EL + k.
    Virtual rows as in module docstring."""
    NSH = cfg.NSH
    vA = (idxA.astype(np.int64) // NSH) * (2 * NSH) + (idxA % NSH)
    vB = (idxB.astype(np.int64) // NSH) * (2 * NSH) + (idxB % NSH) + NSH
    virt = np.concatenate([vA, vB])
    occ = occurrence_rank(virt)
    wave_order = np.argsort(occ, kind="stable")   # edge ids in wave-major order
    wave_sizes = np.bincount(occ)
    return virt, wave_order, wave_sizes


def build_schedule(wave_sizes_all, cfg):
    """wave_sizes_all: list per core of per-wave counts. Returns list of
    (wave_idx, capacity) instruction spans; capacities %128==0, <=TPI."""
    nw = max(len(w) for w in wave_sizes_all)
    mx = np.zeros(nw, np.int64)
    for w in wave_sizes_all:
        mx[:len(w)] = np.maximum(mx[:len(w)], w)
    instr = []
    for w in range(nw):
        s = int(-(-mx[w] // 128) * 128)
        while s > 0:
            t = min(s, cfg.TPI)
            instr.append((w, t))
            s -= t
    return instr


def build_tokens(virt, wave_order, wave_sizes, instr, v_cat, cfg):
    """Build per-core token stream arrays.
    v_cat: [2*EL, M] f32 edge payloads (A then B).
    Returns vtok [TOT, M] f32, idxw [128, TOT/16] int16."""
    TOT = sum(t for _, t in instr)
    M = cfg.M
    vtok = np.zeros((TOT, M), np.float32)
    ids = np.full(TOT, cfg.NV, np.int64)   # dummy row default
    wave_begin = np.r_[0, np.cumsum(wave_sizes)]
    consumed = np.zeros(len(wave_sizes), np.int64)
    pos = 0
    for (w, cap) in instr:
        if w < len(wave_sizes):
            avail = int(wave_sizes[w] - consumed[w])
            take = min(avail, cap)
            if take > 0:
                sl = wave_order[wave_begin[w] + consumed[w]:
                                wave_begin[w] + consumed[w] + take]
                vtok[pos:pos + take] = v_cat[sl]
                ids[pos:pos + take] = virt[sl]
                consumed[w] += take
        pos += cap
    assert pos == TOT
    # wrap ids into the HW token layout per instruction
    idxw = np.empty((8, 16, TOT // 16), np.int16)
    pos = 0
    for (w, cap) in instr:
        blk_ids = ids[pos:pos + cap].astype(np.int16)
        A = blk_ids.reshape(128, cap // 128)
        tok = A.T.reshape(-1)                  # token i value
        blk = tok.reshape(cap // 16, 16).T     # [16, cap/16]
        idxw[:, :, pos // 16:(pos + cap) // 16] = blk[None]
        pos += cap
    return vtok, idxw.reshape(128, TOT // 16)


def host_prep(inputs, cfg):
    """Slice + schedule for all cores. Returns in_maps list and TOT."""
    idxA_all = np.asarray(inputs["e_s2d_dst"])
    idxB_all = np.asarray(inputs["e_d2s_dst"])
    vA_all = np.asarray(inputs["v_s2d"], dtype=np.float32)
    vB_all = np.asarray(inputs["v_d2s"], dtype=np.float32)
    EL = cfg.EL
    per_core = []
    wave_sizes_all = []
    for c in range(cfg.C):
        sl = slice(c * EL, (c + 1) * EL)
        virt, worder, wsizes = core_wave_streams(idxA_all[sl], idxB_all[sl], cfg)
        per_core.append((virt, worder, wsizes, sl))
        wave_sizes_all.append(wsizes)
    instr = build_schedule(wave_sizes_all, cfg)
    TOT = sum(t for _, t in instr)

    # counts per node shard for the bias term
    cntA = np.bincount(idxA_all, minlength=cfg.N).astype(np.float32)
    cntB = np.bincount(idxB_all, minlength=cfg.N).astype(np.float32)

    semb = np.asarray(inputs["src_embed"], dtype=np.float32)
    demb = np.asarray(inputs["dst_embed"], dtype=np.float32)

    def rep(name):
        return np.ascontiguousarray(np.asarray(inputs[name], dtype=np.float32))

    weights = {k: rep(k) for k in [
        "W_src", "b_src", "W_dst", "b_dst", "W_sm", "b_sm", "W_dm", "b_dm",
        "row_W1", "row_b1", "row_g", "row_beta", "row_W2", "row_b2",
        "col_W1", "col_b1", "col_g", "col_beta", "col_W2", "col_b2"]}

    in_maps = []
    for c in range(cfg.C):
        virt, worder, wsizes, sl = per_core[c]
        v_cat = np.concatenate([vA_all[sl], vB_all[sl]], axis=0)
        vtok, idxw = build_tokens(virt, worder, wsizes, instr, v_cat, cfg)
        nsl = slice(c * cfg.NSH, (c + 1) * cfg.NSH)
        m = {
            "vtok": vtok,
            "idxw": idxw,
            "semb": np.ascontiguousarray(semb[nsl]),
            "demb": np.ascontiguousarray(demb[nsl]),
            "sembT": np.ascontiguousarray(semb[nsl].T),
            "dembT": np.ascontiguousarray(demb[nsl].T),
            # [128, 2*ntile]: [p, side*ntile+j] = cnt_side[j*128+p]
            "cntT": np.ascontiguousarray(np.concatenate(
                [cntA[nsl].reshape(-1, 128).T,
                 cntB[nsl].reshape(-1, 128).T], axis=1)),
        }
        m.update(weights)
        in_maps.append(m)
    return in_maps, instr, TOT


# ---------------- device kernel ----------------

def build_kernel(cfg, instr, TOT):
    C, D, M, NSH = cfg.C, cfg.D, cfg.M, cfg.NSH
    nc = bacc.Bacc("TRN2", target_bir_lowering=False, debug=False, num_devices=C)

    vtok = nc.dram_tensor("vtok", [TOT, M], F32, kind="ExternalInput")
    idxw = nc.dram_tensor("idxw", [128, TOT // 16], I16, kind="ExternalInput")
    semb = nc.dram_tensor("semb", [NSH, D], F32, kind="ExternalInput")
    demb = nc.dram_tensor("demb", [NSH, D], F32, kind="ExternalInput")
    sembT = nc.dram_tensor("sembT", [D, NSH], F32, kind="ExternalInput")
    dembT = nc.dram_tensor("dembT", [D, NSH], F32, kind="ExternalInput")
    cnt = nc.dram_tensor("cntT", [128, 2 * (NSH // 128)], F32, kind="ExternalInput")
    wt = {}
    for k, shp in [
        ("W_src", [D, D]), ("b_src", [D]), ("W_dst", [D, D]), ("b_dst", [D]),
        ("W_sm", [M, D]), ("b_sm", [D]), ("W_dm", [M, D]), ("b_dm", [D]),
        ("row_W1", [D, D]), ("row_b1", [D]), ("row_g", [D]), ("row_beta", [D]),
        ("row_W2", [D, D]), ("row_b2", [D]),
        ("col_W1", [D, D]), ("col_b1", [D]), ("col_g", [D]), ("col_beta", [D]),
        ("col_W2", [D, D]), ("col_b2", [D]),
    ]:
        wt[k] = nc.dram_tensor(k, shp, F32, kind="ExternalInput")
    rowo = nc.dram_tensor("rowo", [NSH, D], F32, kind="ExternalOutput")
    colo = nc.dram_tensor("colo", [NSH, D], F32, kind="ExternalOutput")

    with tile.TileContext(nc) as tc:
        with (
            tc.tile_pool(name="const", bufs=1) as const,
            tc.tile_pool(name="zb", bufs=1) as zb,
            tc.tile_pool(name="io", bufs=3) as io,
            tc.tile_pool(name="fin", bufs=3) as fin,
            tc.tile_pool(name="ps", bufs=4, space="PSUM") as ps,
            tc.tile_pool(name="dram", bufs=1, space="DRAM") as dram,
        ):
            table = dram.tile([cfg.TROWS, M], F32)
            rs_out = dram.tile([2 * NSH, M], F32)

            # --- zero the table ---
            zcols = cfg.TROWS * M // 128
            zt = zb.tile([128, zcols], F32)
            nc.gpsimd.memset(zt[:], 0.0)
            nc.sync.dma_start(
                table[:, :].rearrange("n m -> (n m)").rearrange(
                    "(p f) -> p f", p=128), zt[:])

            # --- idx block ---
            idxs = const.tile([128, TOT // 16], I16)
            nc.sync.dma_start(idxs[:], idxw.ap())

            # --- scatter chain ---
            pos = 0
            for (w, cap) in instr:
                src = io.tile([128, cfg.TPI // 128, M], F32, tag="src")
                nc.sync.dma_start(
                    src[:, :cap // 128, :],
                    vtok.ap()[pos:pos + cap, :].rearrange(
                        "(p r) f -> p r f", p=128))
                nc.gpsimd.dma_scatter_add(
                    table[:, :], src[:, :cap // 128, :],
                    idxs[:, pos // 16:(pos + cap) // 16],
                    cap, cap, M)
                pos += cap

            # --- reduce-scatter ---
            nc.gpsimd.collective_compute(
                "ReduceScatter",
                mybir.AluOpType.add,
                replica_groups=[list(range(C))],
                ins=[table[:cfg.NV, :]],
                outs=[rs_out[:, :]],
            )

            # --- constants for finish ---
            ident = const.tile([128, 128], F32)
            make_identity(nc, ident[:])
            eps = const.tile([128, 1], F32)
            nc.vector.memset(eps[:], 1e-5)

            def load_w(name, shp):
                t = const.tile(shp, F32, tag=f"w_{name}")
                nc.sync.dma_start(t[:], wt[name].ap())
                return t

            def load_rep(name):
                t = const.tile([128, D], F32, tag=f"rep_{name}")
                b = wt[name].ap()
                nc.gpsimd.dma_start(
                    t[:], bass.AP(tensor=b.tensor, offset=b.offset,
                                  ap=[[0, 128]] + list(b.ap)))
                return t

            Wm_side = {"col": load_w("W_sm", [M, D]), "row": load_w("W_dm", [M, D])}
            bm_side = {"col": load_rep("b_sm"), "row": load_rep("b_dm")}
            W_side = {"col": load_w("W_dst", [D, D]), "row": load_w("W_src", [D, D])}
            b_side = {"col": load_rep("b_dst"), "row": load_rep("b_src")}
            W1 = {"col": load_w("col_W1", [D, D]), "row": load_w("row_W1", [D, D])}
            b1 = {"col": load_rep("col_b1"), "row": load_rep("row_b1")}
            g_ = {"col": load_rep("col_g"), "row": load_rep("row_g")}
            be = {"col": load_rep("col_beta"), "row": load_rep("row_beta")}
            W2 = {"col": load_w("col_W2", [D, D]), "row": load_w("row_W2", [D, D])}
            b2 = {"col": load_rep("col_b2"), "row": load_rep("row_b2")}

            cnt_t = const.tile([128, 2 * (NSH // 128)], F32)
            nc.sync.dma_start(cnt_t[:], cnt.ap())

            ntile = NSH // 128
            for side, embT_d, emb_d, rs_base, cnt_row, out_d in (
                ("col", dembT, demb, 0, 0, colo),     # A shard: s2d sums
                ("row", sembT, semb, NSH, 1, rowo),   # B shard: d2s sums
            ):
                # rs shard rows [rs_base, rs_base+NSH)
                for j in range(ntile):
                    n0 = j * 128
                    ET = fin.tile([128, 128], F32, tag="ET")
                    nc.sync.dma_start(ET[:], embT_d.ap()[:, n0:n0 + 128])
                    E = fin.tile([128, 128], F32, tag="E")
                    nc.sync.dma_start(E[:], emb_d.ap()[n0:n0 + 128, :])
                    A_t = fin.tile([128, M], F32, tag="A")
                    nc.sync.dma_start(
                        A_t[:], rs_out[rs_base + n0:rs_base + n0 + 128, :])

                    # h = emb @ W + b
                    h_ps = ps.tile([128, D], F32, tag="p1")
                    nc.tensor.matmul(h_ps[:], lhsT=ET[:], rhs=W_side[side][:])
                    h = fin.tile([128, D], F32, tag="h")
                    nc.vector.tensor_add(h[:], h_ps[:], b_side[side][:])

                    # S = A @ Wm + cnt*bm
                    at_ps = ps.tile([M, 128], F32, tag="p2")
                    nc.tensor.transpose(at_ps[:], A_t[:], ident[:])
                    AT = fin.tile([M, 128], F32, tag="AT")
                    nc.vector.tensor_copy(AT[:], at_ps[:])
                    s_ps = ps.tile([128, D], F32, tag="p1")
                    nc.tensor.matmul(s_ps[:], lhsT=AT[:], rhs=Wm_side[side][:])
                    # cnt scalar per node: column cnt_row*ntile + j
                    cb = fin.tile([128, D], F32, tag="cb")
                    nc.vector.tensor_scalar_mul(
                        cb[:], in0=bm_side[side][:],
                        scalar1=cnt_t[:, cnt_row * ntile + j:
                                      cnt_row * ntile + j + 1])
                    S = fin.tile([128, D], F32, tag="S")
                    nc.vector.tensor_add(S[:], s_ps[:], cb[:])

                    # u = h * S
                    u = fin.tile([128, D], F32, tag="u")
                    nc.vector.tensor_mul(u[:], h[:], S[:])

                    # t1 = u @ W1 + b1
                    ut_ps = ps.tile([128, 128], F32, tag="p2")
                    nc.tensor.transpose(ut_ps[:], u[:], ident[:])
                    uT = fin.tile([128, 128], F32, tag="uT")
                    nc.vector.tensor_copy(uT[:], ut_ps[:])
                    t1_ps = ps.tile([128, D], F32, tag="p1")
                    nc.tensor.matmul(t1_ps[:], lhsT=uT[:], rhs=W1[side][:])
                    t1 = fin.tile([128, D], F32, tag="t1")
                    nc.vector.tensor_add(t1[:], t1_ps[:], b1[side][:])

                    # LN(t1) * g + beta
                    stats = fin.tile([128, nc.vector.BN_STATS_DIM], F32, tag="st")
                    nc.vector.bn_stats(stats[:], t1[:])
                    mv = fin.tile([128, nc.vector.BN_AGGR_DIM], F32, tag="mv")
                    nc.vector.bn_aggr(mv[:], stats[:])
                    rstd = fin.tile([128, 1], F32, tag="rs")
                    nc.scalar.activation(
                        rstd[:], mv[:, 1:2],
                        func=mybir.ActivationFunctionType.Sqrt,
                        bias=eps[:], scale=1.0, alpha=0.0)
                    nc.vector.reciprocal(rstd[:], rstd[:])
                    nc.vector.tensor_scalar(
                        t1[:], in0=t1[:], scalar1=mv[:, 0:1], scalar2=rstd[:],
                        op0=mybir.AluOpType.subtract, op1=mybir.AluOpType.mult)
                    nc.vector.tensor_mul(t1[:], t1[:], g_[side][:])
                    nc.vector.tensor_add(t1[:], t1[:], be[side][:])

                    # t2 = ln @ W2 + b2 ; out = emb + t2
                    lt_ps = ps.tile([128, 128], F32, tag="p2")
                    nc.tensor.transpose(lt_ps[:], t1[:], ident[:])
                    lT = fin.tile([128, 128], F32, tag="lT")
                    nc.vector.tensor_copy(lT[:], lt_ps[:])
                    t2_ps = ps.tile([128, D], F32, tag="p1")
                    nc.tensor.matmul(t2_ps[:], lhsT=lT[:], rhs=W2[side][:])
                    ot = fin.tile([128, D], F32, tag="ot")
                    nc.vector.tensor_add(ot[:], t2_ps[:], b2[side][:])
                    nc.vector.tensor_add(ot[:], ot[:], E[:])
                    nc.sync.dma_start(out_d.ap()[n0:n0 + 128, :], ot[:])

    nc.compile()
    return nc


def assemble(results, cfg):
    row = np.concatenate([r["rowo"] for r in results], axis=0)
    col = np.concatenate([r["colo"] for r in results], axis=0)
    return row, col


# ---------------- graded entry point ----------------

_CACHE = {}


def kernel(**inputs):
    cfg = Cfg()
    in_maps, instr, TOT = host_prep(inputs, cfg)
    key = (len(instr), TOT)
    if key not in _CACHE:
        _CACHE[key] = build_kernel(cfg, instr, TOT)
    nc = _CACHE[key]
    from concourse.bass_utils import run_bass_kernel_spmd
    res = run_bass_kernel_spmd(nc, in_maps, core_ids=list(range(cfg.C)))
    return assemble(res.results, cfg)

